# revision 1
# baseline (speedup 1.0000x reference)
"""Trainium2 Bass kernel for a message-aggregation (single-query attention) block.

Computation (per batch row b):
    Q = A @ Wq.T ; K = M @ Wk.T ; V = M @ Wv.T
    attn = softmax(Q . K / sqrt(D))
    out = sigmoid(A @ Wg.T + bg) * LN(attn-weighted V @ Wo.T + bo)

Host-side algebraic restructuring (exact up to fp reassociation):
    scores[b,n] = A[b] @ (Wq.T @ Wk / sqrt(D)) @ M[b,n].T
    agg[b]      = (sum_n attn[b,n] M[b,n]) @ (Wo @ Wv).T + bo
so K and V are never materialized; the device makes a single streaming pass
over `messages` (memory-bound) plus small 512x512 matmuls.

Device dataflow per 128-row batch tile: messages stream in 4-message chunks;
scores (mult+reduce split across DVE/GPSIMD), exp without max subtraction
(scores ~ N(0,1): no overflow), and the attention-weighted sum accumulates in
PSUM via diagonal matmuls; 1/sum(exp) is folded into the PSUM->SBUF
evacuation. All matmuls run in float32r (fast fp32 mode, ~1e-4 rel).

Sharding: pure data parallel over the batch dim across 8 cores; the small
512x512 weights are replicated.
"""

import math
from contextlib import ExitStack

import numpy as np

import concourse.bacc as bacc
import concourse.bass as bass
import concourse.mybir as mybir
import concourse.tile as tile
from concourse.bass_utils import run_bass_kernel_spmd
from concourse.masks import make_identity

B = 4096
N = 32
D = 512
NCORES = 8
BLOC = B // NCORES  # 512
P = 128
NT = BLOC // P  # 4 batch tiles per core
KT = D // P  # 4 contraction tiles
CH = 4  # messages per compute chunk
NCH = N // CH  # 8 chunks per tile
DCH = 8  # messages per DMA (decoupled from compute granularity)
NDCH = N // DCH  # 4 DMA units per tile
SCALE = math.sqrt(D)
LN_EPS = 1e-5

F32 = mybir.dt.float32
F32R = mybir.dt.float32r
ALU = mybir.AluOpType
ACTF = mybir.ActivationFunctionType

# chunks whose score-multiply runs on GPSIMD instead of DVE (DVE does all
# reduces, so it gets fewer multiplies)
POOL_CHUNKS = (0, 2, 3, 5, 7)


def broadcast_mid(ap2d, count):
    """[P, D] AP -> [P, count, D] AP with a step-0 middle dim."""
    return bass.AP(
        tensor=ap2d.tensor,
        offset=ap2d.offset,
        ap=[ap2d.ap[0], [0, count], ap2d.ap[1]],
    )


def build_program(reps=1):
    nc = bacc.Bacc(
        "TRN2",
        target_bir_lowering=False,
        debug=False,
        num_devices=NCORES,
    )

    m_d = nc.dram_tensor("m", [BLOC, N, D], F32R, kind="ExternalInput")
    at_d = nc.dram_tensor("at", [D, BLOC], F32R, kind="ExternalInput")  # A.T
    wqk_d = nc.dram_tensor("wqk", [D, D], F32R, kind="ExternalInput")  # Wq.T Wk/sqrtD
    wgt_d = nc.dram_tensor("wgt", [D, D], F32R, kind="ExternalInput")  # Wg.T
    wvo_d = nc.dram_tensor("wvo", [D, D], F32R, kind="ExternalInput")  # (Wo @ Wv).T
    ones_d = nc.dram_tensor("ones", [1, D], F32R, kind="ExternalInput")
    bg_d = nc.dram_tensor("bg", [1, D], F32R, kind="ExternalInput")
    bo_d = nc.dram_tensor("bo", [1, D], F32R, kind="ExternalInput")
    gamma_d = nc.dram_tensor("gamma", [1, D], F32, kind="ExternalInput")
    beta_d = nc.dram_tensor("beta", [1, D], F32, kind="ExternalInput")
    out_d = nc.dram_tensor("out", [BLOC, D], F32, kind="ExternalOutput")

    with tile.TileContext(nc) as tc, ExitStack() as ctx:
        consts = ctx.enter_context(tc.tile_pool(name="consts", bufs=1))
        atp = ctx.enter_context(tc.tile_pool(name="atp", bufs=KT))
        wts = ctx.enter_context(tc.tile_pool(name="wts", bufs=KT))
        qtp = ctx.enter_context(tc.tile_pool(name="qtp", bufs=NT))
        ggp = ctx.enter_context(tc.tile_pool(name="ggp", bufs=2 * NT))
        mpool = ctx.enter_context(tc.tile_pool(name="mpool", bufs=6))
        prodp = ctx.enter_context(tc.tile_pool(name="prodp", bufs=2))
        smalls = ctx.enter_context(tc.tile_pool(name="smalls", bufs=2))
        diagp = ctx.enter_context(tc.tile_pool(name="diagp", bufs=4))
        bigp = ctx.enter_context(tc.tile_pool(name="bigp", bufs=2))
        lhstp = ctx.enter_context(tc.tile_pool(name="lhstp", bufs=2))
        outp = ctx.enter_context(tc.tile_pool(name="outp", bufs=2))
        ps_a = ctx.enter_context(tc.tile_pool(name="ps_a", bufs=2, space="PSUM"))
        ps_b = ctx.enter_context(tc.tile_pool(name="ps_b", bufs=2, space="PSUM"))
        ps_t = ctx.enter_context(tc.tile_pool(name="ps_t", bufs=2, space="PSUM"))

        # ---- constants -------------------------------------------------
        ident = consts.tile([P, P], F32)
        make_identity(nc, ident[:])

        ones_row = consts.tile([1, D], F32R)
        nc.sync.dma_start(out=ones_row[:], in_=ones_d[:, :])

        eps_t = consts.tile([P, 1], F32)
        nc.vector.memset(eps_t[:], LN_EPS)
        zeros_t = consts.tile([P, 1], F32)
        nc.vector.memset(zeros_t[:], 0.0)

        bg_row = consts.tile([1, D], F32R)
        nc.sync.dma_start(out=bg_row[:], in_=bg_d[:, :])
        bo_row = consts.tile([1, D], F32R)
        nc.sync.dma_start(out=bo_row[:], in_=bo_d[:, :])

        def bcast128(dram_h):
            a = dram_h[0, :]
            return bass.AP(tensor=a.tensor, offset=a.offset, ap=[[0, P]] + list(a.ap))

        gamma_rep = consts.tile([P, D], F32)
        nc.gpsimd.dma_start(out=gamma_rep[:], in_=bcast128(gamma_d))
        beta_rep = consts.tile([P, D], F32)
        nc.gpsimd.dma_start(out=beta_rep[:], in_=bcast128(beta_d))

        for _rep in range(reps):
            # ---- phase 1: Qt = A @ Wqk, gate = sigmoid(A @ Wg.T + bg) ------
            at_t = []
            for k in range(KT):
                t = atp.tile([P, BLOC], F32R, tag="at")
                nc.sync.dma_start(out=t[:], in_=at_d[k * P : (k + 1) * P, :])
                at_t.append(t)

            wqk_t = []
            for k in range(KT):
                t = wts.tile([P, D], F32R, tag="w")
                nc.sync.dma_start(out=t[:], in_=wqk_d[k * P : (k + 1) * P, :])
                wqk_t.append(t)

            qt_t = []
            for m in range(NT):
                pq = ps_a.tile([P, D], F32, tag="psa")
                for k in range(KT):
                    nc.tensor.matmul(
                        pq[:],
                        lhsT=at_t[k][:, m * P : (m + 1) * P],
                        rhs=wqk_t[k][:],
                        start=(k == 0),
                        stop=(k == KT - 1),
                    )
                qt = qtp.tile([P, D], F32, tag="qt")
                nc.scalar.copy(qt[:], pq[:])
                qt_t.append(qt)

            wgt_t = []
            for k in range(KT):
                t = wts.tile([P, D], F32R, tag="w")
                nc.sync.dma_start(out=t[:], in_=wgt_d[k * P : (k + 1) * P, :])
                wgt_t.append(t)

            # gate*gamma and gate*beta, precomputed off the critical path
            gg_t = []
            gb_t = []
            for m in range(NT):
                pg = ps_b.tile([P, D], F32, tag="psb")
                for k in range(KT):
                    nc.tensor.matmul(
                        pg[:],
                        lhsT=at_t[k][:, m * P : (m + 1) * P],
                        rhs=wgt_t[k][:],
                        start=(k == 0),
                        stop=False,
                    )
                nc.tensor.matmul(
                    pg[:],
                    lhsT=ones_row[:, 0:P],
                    rhs=bg_row[:],
                    start=False,
                    stop=True,
                )
                gate = smalls.tile([P, D], F32, tag="gate")
                nc.scalar.activation(gate[:], pg[:], ACTF.Sigmoid)
                gg = ggp.tile([P, D], F32, tag="gg")
                nc.gpsimd.tensor_mul(gg[:], gate[:], gamma_rep[:])
                gg_t.append(gg)
                gb = ggp.tile([P, D], F32, tag="gb")
                nc.gpsimd.tensor_mul(gb[:], gate[:], beta_rep[:])
                gb_t.append(gb)

            wvo_t = []
            for k in range(KT):
                t = wts.tile([P, D], F32R, tag="w")
                nc.sync.dma_start(out=t[:], in_=wvo_d[k * P : (k + 1) * P, :])
                wvo_t.append(t)

            # ---- phase 2: stream message chunks (single pass) ---------------
            # The per-tile tail is emitted one tile late (software pipelining) so
            # the next tile's accumulation isn't blocked behind it in the PE/DVE
            # instruction streams.
            def emit_head(i):
                expd = smalls.tile([P, N], F32, tag="expd")
                pm = ps_a.tile([P, D], F32, tag="psa")
                mu = []
                for u in range(NDCH):
                    t = mpool.tile([P, DCH, D], F32R, tag="m")
                    nc.sync.dma_start(
                        out=t[:],
                        in_=m_d[i * P : (i + 1) * P, u * DCH : (u + 1) * DCH, :],
                    )
                    mu.append(t)
                for c in range(NCH):
                    u, ulo = divmod(c * CH, DCH)
                    mt = mu[u][:, ulo : ulo + CH, :]
                    mt_f32 = mt.bitcast(F32)

                    # partial scores for this chunk
                    prod = prodp.tile([P, CH, D], F32, tag="prod")
                    eng = nc.gpsimd if c in POOL_CHUNKS else nc.vector
                    eng.tensor_mul(prod[:], mt_f32, broadcast_mid(qt_t[i][:], CH))
                    sc_c = smalls.tile([P, CH], F32, tag="sc")
                    nc.vector.tensor_reduce(
                        sc_c[:], prod[:], axis=mybir.AxisListType.X, op=ALU.add
                    )

                    # unnormalized attention weights (softmax w/o max subtraction)
                    nc.scalar.activation(
                        expd[:, c * CH : (c + 1) * CH],
                        sc_c[:],
                        ACTF.Exp,
                        bias=zeros_t[:, 0:1],
                    )

                    # accumulate exp(s_n) * M_n into PSUM via diagonal matmuls
                    for j in range(CH):
                        n = c * CH + j
                        dg = diagp.tile([P, P], F32R, tag="diag")
                        nc.scalar.mul(dg[:], ident[:], expd[:, n : n + 1])
                        nc.tensor.matmul(
                            pm[:],
                            lhsT=dg[:],
                            rhs=mt[:, j, :],
                            start=(n == 0),
                            stop=(n == N - 1),
                        )
                return expd, pm

            def emit_tail(i, expd, pm):
                # softmax denominator; fold 1/sum into the PSUM evacuation
                sumexp = smalls.tile([P, 1], F32, tag="sumexp")
                nc.vector.tensor_reduce(
                    sumexp[:], expd[:], axis=mybir.AxisListType.X, op=ALU.add
                )
                rsum = smalls.tile([P, 1], F32, tag="rsum")
                nc.vector.reciprocal(rsum[:], sumexp[:])
                magg = bigp.tile([P, D], F32, tag="magg")
                nc.scalar.mul(magg[:], pm[:], rsum[:, 0:1])

                # transpose m_agg so it can be the stationary operand
                pt = ps_t.tile([P, KT, P], F32, tag="pst")
                for j in range(KT):
                    nc.tensor.transpose(pt[:, j, :], magg[:, j * P : (j + 1) * P], ident[:])
                maggT = lhstp.tile([P, KT, P], F32R, tag="lhst")
                for j in range(KT):
                    nc.scalar.copy(maggT[:, j, :], pt[:, j, :])

                # agg = m_agg @ (Wo Wv).T + bo
                pa = ps_b.tile([P, D], F32, tag="psb")
                for j in range(KT):
                    nc.tensor.matmul(
                        pa[:],
                        lhsT=maggT[:, j, :],
                        rhs=wvo_t[j][:],
                        start=(j == 0),
                        stop=False,
                    )
                nc.tensor.matmul(
                    pa[:],
                    lhsT=ones_row[:, 0:P],
                    rhs=bo_row[:],
                    start=False,
                    stop=True,
                )

                # LayerNorm over d
                stats = smalls.tile([P, nc.vector.BN_STATS_DIM], F32, tag="stats")
                nc.vector.bn_stats(stats[:], pa[:])
                mv = smalls.tile([P, nc.vector.BN_AGGR_DIM], F32, tag="mv")
                nc.vector.bn_aggr(mv[:], stats[:])
                sq = smalls.tile([P, 1], F32, tag="sq")
                nc.scalar.activation(sq[:], mv[:, 1:2], ACTF.Sqrt, bias=eps_t[:, 0:1])
                rstd = smalls.tile([P, 1], F32, tag="rstd")
                nc.vector.reciprocal(rstd[:], sq[:])
                negmr = smalls.tile([P, 1], F32, tag="negmr")
                nc.vector.tensor_scalar(
                    negmr[:],
                    mv[:, 0:1],
                    scalar1=rstd[:, 0:1],
                    scalar2=-1.0,
                    op0=ALU.mult,
                    op1=ALU.mult,
                )
                normed = outp.tile([P, D], F32, tag="normed")
                nc.scalar.activation(
                    normed[:], pa[:], ACTF.Identity, bias=negmr[:, 0:1], scale=rstd[:, 0:1]
                )

                # out = (gate*gamma)*normed + gate*beta
                o = outp.tile([P, D], F32, tag="out")
                nc.vector.tensor_mul(o[:], normed[:], gg_t[i][:])
                nc.vector.tensor_add(o[:], o[:], gb_t[i][:])
                nc.sync.dma_start(out=out_d[i * P : (i + 1) * P, :], in_=o[:])

            pending = None
            for i in range(NT):
                head = emit_head(i)
                if pending is not None:
                    emit_tail(i - 1, *pending)
                pending = head
            emit_tail(NT - 1, *pending)

    nc.compile()
    return nc


_CACHED_NC = None


def _get_program():
    global _CACHED_NC
    if _CACHED_NC is None:
        _CACHED_NC = build_program()
    return _CACHED_NC


def make_in_maps(agent_hidden, messages, Wq, Wk, Wv, Wo, bo, gamma, beta, Wg, bg):
    A = np.asarray(agent_hidden, np.float32)
    M = np.asarray(messages, np.float32)
    wq = np.asarray(Wq, np.float64)
    wk = np.asarray(Wk, np.float64)
    wv = np.asarray(Wv, np.float64)
    wo = np.asarray(Wo, np.float64)
    wg = np.asarray(Wg, np.float32)

    wqk = np.ascontiguousarray(((wq.T @ wk) / SCALE).astype(np.float32))
    wvo = np.ascontiguousarray((wo @ wv).T.astype(np.float32))
    wgt = np.ascontiguousarray(wg.T)
    bg_r = np.ascontiguousarray(np.asarray(bg, np.float32).reshape(1, D))
    bo_r = np.ascontiguousarray(np.asarray(bo, np.float32).reshape(1, D))
    gamma_r = np.ascontiguousarray(np.asarray(gamma, np.float32).reshape(1, D))
    beta_r = np.ascontiguousarray(np.asarray(beta, np.float32).reshape(1, D))

    in_maps = []
    for c in range(NCORES):
        sl = slice(c * BLOC, (c + 1) * BLOC)
        in_maps.append(
            {
                "m": np.ascontiguousarray(M[sl]),
                "at": np.ascontiguousarray(A[sl].T),
                "wqk": wqk,
                "wgt": wgt,
                "wvo": wvo,
                "ones": np.ones((1, D), np.float32),
                "bg": bg_r,
                "bo": bo_r,
                "gamma": gamma_r,
                "beta": beta_r,
            }
        )
    return in_maps


def kernel(**inputs) -> np.ndarray:
    nc = _get_program()
    in_maps = make_in_maps(**inputs)
    res = run_bass_kernel_spmd(nc, in_maps, core_ids=list(range(NCORES)))
    return np.concatenate([r["out"] for r in res.results], axis=0)



# revision 4
# speedup vs baseline: 1.4440x; 1.4440x over previous
"""Trainium2 Bass kernel for a message-aggregation (single-query attention) block.

Computation (per batch row b):
    Q = A @ Wq.T ; K = M @ Wk.T ; V = M @ Wv.T
    attn = softmax(Q . K / sqrt(D))
    out = sigmoid(A @ Wg.T + bg) * LN(attn-weighted V @ Wo.T + bo)

Host-side algebraic restructuring (exact up to fp reassociation):
    scores[b,n] = Qt[b] . M[b,n]          with Qt = A @ (Wq.T @ Wk) / sqrt(D)
    agg[b]      = (sum_n attn[b,n] M[b,n]) @ (Wo @ Wv).T + bo
    out         = gg * LN_nogamma(agg) + gb   with gg = gate*gamma, gb = gate*beta
Qt, gg, gb are cheap O(B*D) host precomputes; K and V are never materialized.
Messages are shipped to the device in bf16 (halves the HBM stream; the score
accumulation and the attention-weighted sum still accumulate in fp32).

Device dataflow per 128-row batch tile (three-stage software pipeline):
  scores(i):  messages stream in 16-message units; unit 0's scores via fused
              DVE tensor_tensor_reduce (one pass, fp32 accum), unit 1 via one
              2x-mode bf16 multiply + ScalarE Copy-with-accum reductions;
              exp (no max subtraction; scores ~ N(0,1)) + its sum in one
              ScalarE instruction.
  accum(i-1): per-message diag(exp) built by DVE tensor_scalar (4x mode) and
              accumulated into PSUM via TensorE diag matmuls (bf16).
  tail(i-2):  1/sumexp folded into the PSUM evacuation, transpose, (Wo Wv).T
              matmul + bias, LayerNorm, gate multiply, store.

Sharding: pure data parallel over the batch dim across 8 cores; the small
512x512 weights are replicated.
"""

import math
from contextlib import ExitStack

import ml_dtypes
import numpy as np

import concourse.bacc as bacc
import concourse.bass as bass
import concourse.mybir as mybir
import concourse.tile as tile
from concourse.bass_utils import run_bass_kernel_spmd
from concourse.masks import make_identity

B = 4096
N = 32
D = 512
NCORES = 8
BLOC = B // NCORES  # 512
P = 128
NT = BLOC // P  # 4 batch tiles per core
KT = D // P  # 4 contraction tiles
U = 16  # messages per DMA unit / score sub-block
NU = N // U  # 2 units per tile
SCALE = math.sqrt(D)
LN_EPS = 1e-5

F32 = mybir.dt.float32
BF16 = mybir.dt.bfloat16
ALU = mybir.AluOpType
ACTF = mybir.ActivationFunctionType


def broadcast_mid(ap2d, count):
    """[P, D] AP -> [P, count, D] AP with a step-0 middle dim."""
    return bass.AP(
        tensor=ap2d.tensor,
        offset=ap2d.offset,
        ap=[ap2d.ap[0], [0, count], ap2d.ap[1]],
    )


def build_program():
    nc = bacc.Bacc(
        "TRN2",
        target_bir_lowering=False,
        debug=False,
        num_devices=NCORES,
    )

    m_d = nc.dram_tensor("m", [BLOC, N, D], BF16, kind="ExternalInput")
    qt_d = nc.dram_tensor("qt", [BLOC, D], BF16, kind="ExternalInput")
    gg_d = nc.dram_tensor("gg", [BLOC, D], F32, kind="ExternalInput")
    gb_d = nc.dram_tensor("gb", [BLOC, D], F32, kind="ExternalInput")
    wvo_d = nc.dram_tensor("wvo", [D, D], BF16, kind="ExternalInput")  # (Wo @ Wv).T
    ones_d = nc.dram_tensor("ones", [1, D], BF16, kind="ExternalInput")
    bo_d = nc.dram_tensor("bo", [1, D], BF16, kind="ExternalInput")
    out_d = nc.dram_tensor("out", [BLOC, D], F32, kind="ExternalOutput")

    with tile.TileContext(nc) as tc, ExitStack() as ctx:
        consts = ctx.enter_context(tc.tile_pool(name="consts", bufs=1))
        mpool = ctx.enter_context(tc.tile_pool(name="mpool", bufs=6))
        qtp = ctx.enter_context(tc.tile_pool(name="qtp", bufs=NT))
        ggp = ctx.enter_context(tc.tile_pool(name="ggp", bufs=2 * NT))
        wts = ctx.enter_context(tc.tile_pool(name="wts", bufs=KT))
        prodp = ctx.enter_context(tc.tile_pool(name="prodp", bufs=4))
        prod16p = ctx.enter_context(tc.tile_pool(name="prod16p", bufs=2))
        dumpp = ctx.enter_context(tc.tile_pool(name="dumpp", bufs=2))
        scp = ctx.enter_context(tc.tile_pool(name="scp", bufs=3))
        diagp = ctx.enter_context(tc.tile_pool(name="diagp", bufs=8))
        smalls = ctx.enter_context(tc.tile_pool(name="smalls", bufs=2))
        bigp = ctx.enter_context(tc.tile_pool(name="bigp", bufs=2))
        lhstp = ctx.enter_context(tc.tile_pool(name="lhstp", bufs=2))
        outp = ctx.enter_context(tc.tile_pool(name="outp", bufs=2))
        ps_m = ctx.enter_context(tc.tile_pool(name="ps_m", bufs=2, space="PSUM"))
        ps_t = ctx.enter_context(tc.tile_pool(name="ps_t", bufs=2, space="PSUM"))
        ps_a = ctx.enter_context(tc.tile_pool(name="ps_a", bufs=2, space="PSUM"))

        # ---- message stream: issue all DMAs up front (buffer-recycle sems
        # gate the later units); sync/HWDGE ring is dedicated to messages +
        # the output stores.
        m_tiles = []  # [tile][unit]
        for i in range(NT):
            row = []
            for u in range(NU):
                t = mpool.tile([P, U, D], BF16, tag="m")
                nc.sync.dma_start(
                    out=t[:],
                    in_=m_d[i * P : (i + 1) * P, u * U : (u + 1) * U, :],
                )
                row.append(t)
            m_tiles.append(row)

        # ---- small inputs on the scalar/HWDGE ring (doesn't queue behind
        # the message stream).
        qt_t = []
        for i in range(NT):
            t = qtp.tile([P, D], BF16, tag="qt")
            nc.scalar.dma_start(out=t[:], in_=qt_d[i * P : (i + 1) * P, :])
            qt_t.append(t)

        wvo_t = []
        for k in range(KT):
            t = wts.tile([P, D], BF16, tag="w")
            nc.scalar.dma_start(out=t[:], in_=wvo_d[k * P : (k + 1) * P, :])
            wvo_t.append(t)

        ones_row = consts.tile([1, D], BF16)
        nc.scalar.dma_start(out=ones_row[:], in_=ones_d[:, :])
        bo_row = consts.tile([1, D], BF16)
        nc.scalar.dma_start(out=bo_row[:], in_=bo_d[:, :])

        gg_t = []
        gb_t = []
        for i in range(NT):
            t = ggp.tile([P, D], F32, tag="gg")
            nc.scalar.dma_start(out=t[:], in_=gg_d[i * P : (i + 1) * P, :])
            gg_t.append(t)
            t = ggp.tile([P, D], F32, tag="gb")
            nc.scalar.dma_start(out=t[:], in_=gb_d[i * P : (i + 1) * P, :])
            gb_t.append(t)

        # ---- constants ------------------------------------------------
        ident = consts.tile([P, P], BF16)
        make_identity(nc, ident[:])
        eps_t = consts.tile([P, 1], F32)
        nc.vector.memset(eps_t[:], LN_EPS)

        # ---- pipeline stages ------------------------------------------
        def emit_scores(i):
            """Fill sc[P, N] (fp32) and expd[P, N]; returns per-tile state."""
            sc = scp.tile([P, N], F32, tag="sc")
            # unit 0: fused multiply+reduce on DVE, one instruction/message
            mt0 = m_tiles[i][0]
            for j in range(U):
                prod = prodp.tile([P, D], BF16, tag="prod")
                nc.vector.scalar_tensor_tensor(
                    out=prod[:],
                    in0=mt0[:, j, :],
                    scalar=0.0,
                    in1=qt_t[i][:],
                    op0=ALU.bypass,
                    op1=ALU.mult,
                    accum_out=sc[:, j : j + 1],
                )
            # unit 1: one 2x-mode bf16 multiply, reductions on ScalarE
            mt1 = m_tiles[i][1]
            prod16 = prod16p.tile([P, U, D], BF16, tag="prod16")
            nc.vector.tensor_mul(prod16[:], mt1[:], broadcast_mid(qt_t[i][:], U))
            for j in range(U):
                dump = dumpp.tile([P, D], BF16, tag="dump")
                nc.scalar.activation(
                    dump[:],
                    prod16[:, j, :],
                    ACTF.Copy,
                    accum_out=sc[:, U + j : U + j + 1],
                )
            # exp (no max subtraction) + softmax denominator in one shot
            expd = scp.tile([P, N], F32, tag="expd")
            sumexp = smalls.tile([P, 1], F32, tag="sumexp")
            nc.scalar.activation(expd[:], sc[:], ACTF.Exp, accum_out=sumexp[:])
            return expd, sumexp

        def emit_accum(i, state):
            """exp-weighted message sum into PSUM via diag matmuls."""
            expd, _ = state
            pm = ps_m.tile([P, D], F32, tag="pm")
            for n in range(N):
                u, j = divmod(n, U)
                dg = diagp.tile([P, P], BF16, tag="diag")
                nc.vector.tensor_scalar(
                    out=dg[:],
                    in0=ident[:],
                    scalar1=expd[:, n : n + 1],
                    scalar2=None,
                    op0=ALU.mult,
                )
                nc.tensor.matmul(
                    pm[:],
                    lhsT=dg[:],
                    rhs=m_tiles[i][u][:, j, :],
                    start=(n == 0),
                    stop=(n == N - 1),
                )
            return pm

        def emit_tail(i, state, pm):
            expd, sumexp = state
            rsum = smalls.tile([P, 1], F32, tag="rsum")
            nc.vector.reciprocal(rsum[:], sumexp[:])
            # fold 1/sum(exp) into the PSUM evacuation
            magg = bigp.tile([P, D], BF16, tag="magg")
            nc.scalar.mul(magg[:], pm[:], rsum[:, 0:1])

            # transpose m_agg so it can be the stationary operand
            pt = ps_t.tile([P, KT, P], BF16, tag="pt")
            for j in range(KT):
                nc.tensor.transpose(pt[:, j, :], magg[:, j * P : (j + 1) * P], ident[:])
            maggT = lhstp.tile([P, KT, P], BF16, tag="lhst")
            for j in range(KT):
                nc.scalar.copy(maggT[:, j, :], pt[:, j, :])

            # agg = m_agg @ (Wo Wv).T + bo
            pa = ps_a.tile([P, D], F32, tag="pa")
            for j in range(KT):
                nc.tensor.matmul(
                    pa[:],
                    lhsT=maggT[:, j, :],
                    rhs=wvo_t[j][:],
                    start=(j == 0),
                    stop=False,
                )
            nc.tensor.matmul(
                pa[:],
                lhsT=ones_row[:, 0:P],
                rhs=bo_row[:],
                start=False,
                stop=True,
            )

            # LayerNorm over d (gamma/beta folded into gg/gb on host)
            stats = smalls.tile([P, nc.vector.BN_STATS_DIM], F32, tag="stats")
            nc.vector.bn_stats(stats[:], pa[:])
            mv = smalls.tile([P, nc.vector.BN_AGGR_DIM], F32, tag="mv")
            nc.vector.bn_aggr(mv[:], stats[:])
            sq = smalls.tile([P, 1], F32, tag="sq")
            nc.scalar.activation(sq[:], mv[:, 1:2], ACTF.Sqrt, bias=eps_t[:, 0:1])
            rstd = smalls.tile([P, 1], F32, tag="rstd")
            nc.vector.reciprocal(rstd[:], sq[:])
            negmr = smalls.tile([P, 1], F32, tag="negmr")
            nc.vector.tensor_scalar(
                negmr[:],
                mv[:, 0:1],
                scalar1=rstd[:, 0:1],
                scalar2=-1.0,
                op0=ALU.mult,
                op1=ALU.mult,
            )
            normed = outp.tile([P, D], F32, tag="normed")
            nc.scalar.activation(
                normed[:], pa[:], ACTF.Identity, bias=negmr[:, 0:1], scale=rstd[:, 0:1]
            )

            # out = gg*normed + gb  (on GPSIMD; DVE/ACT are the hot engines)
            o = outp.tile([P, D], F32, tag="out")
            nc.gpsimd.tensor_mul(o[:], normed[:], gg_t[i][:])
            nc.gpsimd.tensor_add(o[:], o[:], gb_t[i][:])
            nc.sync.dma_start(out=out_d[i * P : (i + 1) * P, :], in_=o[:])

        states = [None] * NT
        pms = [None] * NT
        for s in range(NT + 2):
            if s < NT:
                states[s] = emit_scores(s)
            if 0 <= s - 2 < NT:
                emit_tail(s - 2, states[s - 2], pms[s - 2])
            if 0 <= s - 1 < NT:
                pms[s - 1] = emit_accum(s - 1, states[s - 1])

    nc.compile()
    return nc


_CACHED_NC = None


def _get_program():
    global _CACHED_NC
    if _CACHED_NC is None:
        _CACHED_NC = build_program()
    return _CACHED_NC


def make_in_maps(agent_hidden, messages, Wq, Wk, Wv, Wo, bo, gamma, beta, Wg, bg):
    BF = ml_dtypes.bfloat16
    A = np.asarray(agent_hidden, np.float32)
    M = np.asarray(messages, np.float32)
    wq = np.asarray(Wq, np.float64)
    wk = np.asarray(Wk, np.float64)
    wv = np.asarray(Wv, np.float64)
    wo = np.asarray(Wo, np.float64)

    wqk = ((wq.T @ wk) / SCALE).astype(np.float32)
    qt = (A @ wqk).astype(BF)
    gate = 1.0 / (1.0 + np.exp(-(A @ np.asarray(Wg, np.float32).T + np.asarray(bg, np.float32))))
    gg = (gate * np.asarray(gamma, np.float32)).astype(np.float32)
    gb = (gate * np.asarray(beta, np.float32)).astype(np.float32)
    wvo = np.ascontiguousarray((wo @ wv).T.astype(BF))
    m16 = M.astype(BF)

    bo_r = np.asarray(bo, np.float32).astype(BF).reshape(1, D)
    ones_r = np.ones((1, D), BF)

    in_maps = []
    for c in range(NCORES):
        sl = slice(c * BLOC, (c + 1) * BLOC)
        in_maps.append(
            {
                "m": np.ascontiguousarray(m16[sl]),
                "qt": np.ascontiguousarray(qt[sl]),
                "gg": np.ascontiguousarray(gg[sl]),
                "gb": np.ascontiguousarray(gb[sl]),
                "wvo": wvo,
                "ones": ones_r,
                "bo": bo_r,
            }
        )
    return in_maps


def kernel(**inputs) -> np.ndarray:
    nc = _get_program()
    in_maps = make_in_maps(**inputs)
    res = run_bass_kernel_spmd(nc, in_maps, core_ids=list(range(NCORES)))
    return np.concatenate([r["out"] for r in res.results], axis=0)


# revision 5
# speedup vs baseline: 1.4769x; 1.0228x over previous
"""Trainium2 Bass kernel for a message-aggregation (single-query attention) block.

Computation (per batch row b):
    Q = A @ Wq.T ; K = M @ Wk.T ; V = M @ Wv.T
    attn = softmax(Q . K / sqrt(D))
    out = sigmoid(A @ Wg.T + bg) * LN(attn-weighted V @ Wo.T + bo)

Host-side algebraic restructuring (exact up to fp reassociation):
    scores[b,n] = Qt[b] . M[b,n]          with Qt = A @ (Wq.T @ Wk) / sqrt(D)
    agg[b]      = (sum_n attn[b,n] M[b,n]) @ (Wo @ Wv).T + bo
    out         = gg * LN_nogamma(agg) + gb   with gg = gate*gamma, gb = gate*beta
Qt, gg, gb are cheap O(B*D) host precomputes; K and V are never materialized.
Messages are shipped to the device in fp16 (halves the HBM stream; score and
attention-weighted sums still accumulate in fp32).

Device dataflow per 128-row batch tile (three-stage software pipeline):
  scores(i):  messages stream in 16-message units; unit 0's scores via fused
              DVE scalar_tensor_tensor (multiply + fp32-accumulated sum in one
              pass), unit 1 via 2x-mode fp16 multiplies + ScalarE
              Copy-with-accumulate reductions (balances DVE vs ScalarE);
              exp (no max subtraction; scores ~ N(0,1)) + its sum in one
              ScalarE instruction.
  accum(i-1): diag(exp) matrices built 16-at-a-time on GPSIMD (broadcast
              access patterns), accumulated into PSUM via TensorE diag
              matmuls (fp16, fp32 accumulation).
  tail(i-2):  1/sumexp folded into the PSUM evacuation, transpose, (Wo Wv).T
              matmul + bias, LayerNorm, gate multiply (GPSIMD), fp16 store
              (upcast on host).

Sharding: pure data parallel over the batch dim across 8 cores; the small
512x512 weights are replicated.
"""

import math
from contextlib import ExitStack

import numpy as np

import concourse.bacc as bacc
import concourse.bass as bass
import concourse.mybir as mybir
import concourse.tile as tile
from concourse.bass_utils import run_bass_kernel_spmd
from concourse.masks import make_identity

B = 4096
N = 32
D = 512
NCORES = 8
BLOC = B // NCORES  # 512
P = 128
NT = BLOC // P  # 4 batch tiles per core
KT = D // P  # 4 contraction tiles
U = 16  # messages per DMA unit / score sub-block
NU = N // U  # 2 units per tile
SCALE = math.sqrt(D)
LN_EPS = 1e-5

F32 = mybir.dt.float32
F16 = mybir.dt.float16
ALU = mybir.AluOpType
ACTF = mybir.ActivationFunctionType


def broadcast_mid(ap2d, count):
    """[P, X] AP -> [P, count, X] AP with a step-0 middle dim."""
    return bass.AP(
        tensor=ap2d.tensor,
        offset=ap2d.offset,
        ap=[ap2d.ap[0], [0, count], ap2d.ap[1]],
    )


def broadcast_last(ap2d, count):
    """[P, X] AP -> [P, X, count] AP with a step-0 last dim."""
    return bass.AP(
        tensor=ap2d.tensor,
        offset=ap2d.offset,
        ap=[ap2d.ap[0], ap2d.ap[1], [0, count]],
    )


def build_program():
    nc = bacc.Bacc(
        "TRN2",
        target_bir_lowering=False,
        debug=False,
        num_devices=NCORES,
    )

    m_d = nc.dram_tensor("m", [BLOC, N, D], F16, kind="ExternalInput")
    qt_d = nc.dram_tensor("qt", [BLOC, D], F16, kind="ExternalInput")
    gg_d = nc.dram_tensor("gg", [BLOC, D], F16, kind="ExternalInput")
    gb_d = nc.dram_tensor("gb", [BLOC, D], F16, kind="ExternalInput")
    wvo_d = nc.dram_tensor("wvo", [D, D], F16, kind="ExternalInput")  # (Wo @ Wv).T
    ones_d = nc.dram_tensor("ones", [1, D], F16, kind="ExternalInput")
    bo_d = nc.dram_tensor("bo", [1, D], F16, kind="ExternalInput")
    out_d = nc.dram_tensor("out", [BLOC, D], F16, kind="ExternalOutput")

    with tile.TileContext(nc) as tc, ExitStack() as ctx:
        consts = ctx.enter_context(tc.tile_pool(name="consts", bufs=1))
        mpool = ctx.enter_context(tc.tile_pool(name="mpool", bufs=6))
        qtp = ctx.enter_context(tc.tile_pool(name="qtp", bufs=NT))
        ggp = ctx.enter_context(tc.tile_pool(name="ggp", bufs=2 * NT))
        wts = ctx.enter_context(tc.tile_pool(name="wts", bufs=KT))
        prodp = ctx.enter_context(tc.tile_pool(name="prodp", bufs=4))
        prod16p = ctx.enter_context(tc.tile_pool(name="prod16p", bufs=4))
        dumpp = ctx.enter_context(tc.tile_pool(name="dumpp", bufs=2))
        scp = ctx.enter_context(tc.tile_pool(name="scp", bufs=3))
        diagp = ctx.enter_context(tc.tile_pool(name="diagp", bufs=4))
        smalls = ctx.enter_context(tc.tile_pool(name="smalls", bufs=3))
        bigp = ctx.enter_context(tc.tile_pool(name="bigp", bufs=2))
        lhstp = ctx.enter_context(tc.tile_pool(name="lhstp", bufs=2))
        outp = ctx.enter_context(tc.tile_pool(name="outp", bufs=2))
        ps_m = ctx.enter_context(tc.tile_pool(name="ps_m", bufs=2, space="PSUM"))
        ps_t = ctx.enter_context(tc.tile_pool(name="ps_t", bufs=2, space="PSUM"))
        ps_a = ctx.enter_context(tc.tile_pool(name="ps_a", bufs=2, space="PSUM"))

        # ---- message stream: issue all DMAs up front (buffer-recycle sems
        # gate the later units); sync/HWDGE ring is dedicated to messages +
        # the output stores.
        m_tiles = []  # [tile][unit]
        for i in range(NT):
            row = []
            for u in range(NU):
                t = mpool.tile([P, U, D], F16, tag="m")
                nc.sync.dma_start(
                    out=t[:],
                    in_=m_d[i * P : (i + 1) * P, u * U : (u + 1) * U, :],
                )
                row.append(t)
            m_tiles.append(row)

        # ---- small inputs on the scalar/HWDGE ring (doesn't queue behind
        # the message stream).
        qt_t = []
        for i in range(NT):
            t = qtp.tile([P, D], F16, tag="qt")
            nc.scalar.dma_start(out=t[:], in_=qt_d[i * P : (i + 1) * P, :])
            qt_t.append(t)

        wvo_t = []
        for k in range(KT):
            t = wts.tile([P, D], F16, tag="w")
            nc.scalar.dma_start(out=t[:], in_=wvo_d[k * P : (k + 1) * P, :])
            wvo_t.append(t)

        ones_row = consts.tile([1, D], F16)
        nc.scalar.dma_start(out=ones_row[:], in_=ones_d[:, :])
        bo_row = consts.tile([1, D], F16)
        nc.scalar.dma_start(out=bo_row[:], in_=bo_d[:, :])

        gg_t = []
        gb_t = []
        for i in range(NT):
            t = ggp.tile([P, D], F16, tag="gg")
            nc.scalar.dma_start(out=t[:], in_=gg_d[i * P : (i + 1) * P, :])
            gg_t.append(t)
            t = ggp.tile([P, D], F16, tag="gb")
            nc.scalar.dma_start(out=t[:], in_=gb_d[i * P : (i + 1) * P, :])
            gb_t.append(t)

        # ---- constants ------------------------------------------------
        ident = consts.tile([P, P], F16)
        make_identity(nc, ident[:])
        eps_t = consts.tile([P, 1], F32)
        nc.vector.memset(eps_t[:], LN_EPS)

        # ---- pipeline stages ------------------------------------------
        def emit_scores(i):
            """Fill sc[P, N] (fp32) and expd[P, N] (fp16) + sumexp."""
            sc = scp.tile([P, N], F32, tag="sc")
            # unit 1 multiplies first (2x mode) so ScalarE can start reducing
            # while DVE works through unit 0's fused score ops.
            mt1 = m_tiles[i][1]
            prods = []
            for h in range(2):
                p16 = prod16p.tile([P, U // 2, D], F16, tag="prod16")
                nc.vector.tensor_mul(
                    p16[:],
                    mt1[:, h * (U // 2) : (h + 1) * (U // 2), :],
                    broadcast_mid(qt_t[i][:], U // 2),
                )
                prods.append(p16)
            for j in range(U):
                h, jj = divmod(j, U // 2)
                dump = dumpp.tile([P, D], F16, tag="dump")
                nc.scalar.activation(
                    dump[:],
                    prods[h][:, jj, :],
                    ACTF.Copy,
                    accum_out=sc[:, U + j : U + j + 1],
                )
            # unit 0: fused multiply+reduce on DVE, one instruction/message
            mt0 = m_tiles[i][0]
            for j in range(U):
                prod = prodp.tile([P, D], F16, tag="prod")
                nc.vector.scalar_tensor_tensor(
                    out=prod[:],
                    in0=mt0[:, j, :],
                    scalar=0.0,
                    in1=qt_t[i][:],
                    op0=ALU.bypass,
                    op1=ALU.mult,
                    accum_out=sc[:, j : j + 1],
                )
            # exp (no max subtraction) + softmax denominator in one shot
            expd = scp.tile([P, N], F16, tag="expd")
            sumexp = smalls.tile([P, 1], F32, tag="sumexp")
            nc.scalar.activation(expd[:], sc[:], ACTF.Exp, accum_out=sumexp[:])
            return expd, sumexp

        def emit_accum(i, state):
            """exp-weighted message sum into PSUM via diag matmuls."""
            expd, _ = state
            pm = ps_m.tile([P, D], F32, tag="pm")
            for u in range(NU):
                dg = diagp.tile([P, U, P], F16, tag="diag")
                nc.gpsimd.tensor_tensor(
                    dg[:],
                    broadcast_mid(ident[:], U),
                    broadcast_last(expd[:, u * U : (u + 1) * U], P),
                    op=ALU.mult,
                )
                for j in range(U):
                    n = u * U + j
                    nc.tensor.matmul(
                        pm[:],
                        lhsT=dg[:, j, :],
                        rhs=m_tiles[i][u][:, j, :],
                        start=(n == 0),
                        stop=(n == N - 1),
                    )
            return pm

        def emit_tail(i, state, pm):
            expd, sumexp = state
            rsum = smalls.tile([P, 1], F32, tag="rsum")
            nc.vector.reciprocal(rsum[:], sumexp[:])
            # fold 1/sum(exp) into the PSUM evacuation
            magg = bigp.tile([P, D], F16, tag="magg")
            nc.scalar.mul(magg[:], pm[:], rsum[:, 0:1])

            # transpose m_agg so it can be the stationary operand
            pt = ps_t.tile([P, KT, P], F16, tag="pt")
            for j in range(KT):
                nc.tensor.transpose(pt[:, j, :], magg[:, j * P : (j + 1) * P], ident[:])
            maggT = lhstp.tile([P, KT, P], F16, tag="lhst")
            for j in range(KT):
                nc.scalar.copy(maggT[:, j, :], pt[:, j, :])

            # agg = m_agg @ (Wo Wv).T + bo
            pa = ps_a.tile([P, D], F32, tag="pa")
            for j in range(KT):
                nc.tensor.matmul(
                    pa[:],
                    lhsT=maggT[:, j, :],
                    rhs=wvo_t[j][:],
                    start=(j == 0),
                    stop=False,
                )
            nc.tensor.matmul(
                pa[:],
                lhsT=ones_row[:, 0:P],
                rhs=bo_row[:],
                start=False,
                stop=True,
            )

            # LayerNorm over d (gamma/beta folded into gg/gb on host)
            stats = smalls.tile([P, nc.vector.BN_STATS_DIM], F32, tag="stats")
            nc.vector.bn_stats(stats[:], pa[:])
            mv = smalls.tile([P, nc.vector.BN_AGGR_DIM], F32, tag="mv")
            nc.vector.bn_aggr(mv[:], stats[:])
            sq = smalls.tile([P, 1], F32, tag="sq")
            nc.scalar.activation(sq[:], mv[:, 1:2], ACTF.Sqrt, bias=eps_t[:, 0:1])
            rstd = smalls.tile([P, 1], F32, tag="rstd")
            nc.vector.reciprocal(rstd[:], sq[:])
            negmr = smalls.tile([P, 1], F32, tag="negmr")
            nc.vector.tensor_scalar(
                negmr[:],
                mv[:, 0:1],
                scalar1=rstd[:, 0:1],
                scalar2=-1.0,
                op0=ALU.mult,
                op1=ALU.mult,
            )
            normed = outp.tile([P, D], F16, tag="normed")
            nc.scalar.activation(
                normed[:], pa[:], ACTF.Identity, bias=negmr[:, 0:1], scale=rstd[:, 0:1]
            )

            # out = gg*normed + gb  (on GPSIMD; DVE/ACT are the hot engines)
            o = outp.tile([P, D], F16, tag="out")
            nc.gpsimd.tensor_mul(o[:], normed[:], gg_t[i][:])
            nc.gpsimd.tensor_add(o[:], o[:], gb_t[i][:])
            nc.sync.dma_start(out=out_d[i * P : (i + 1) * P, :], in_=o[:])

        states = [None] * NT
        pms = [None] * NT
        for s in range(NT + 2):
            if s < NT:
                states[s] = emit_scores(s)
            if 0 <= s - 1 < NT:
                pms[s - 1] = emit_accum(s - 1, states[s - 1])
            if 0 <= s - 2 < NT:
                emit_tail(s - 2, states[s - 2], pms[s - 2])

    nc.compile()
    return nc


_CACHED_NC = None


def _get_program():
    global _CACHED_NC
    if _CACHED_NC is None:
        _CACHED_NC = build_program()
    return _CACHED_NC


def make_in_maps(agent_hidden, messages, Wq, Wk, Wv, Wo, bo, gamma, beta, Wg, bg):
    A = np.asarray(agent_hidden, np.float32)
    M = np.asarray(messages, np.float32)
    wq = np.asarray(Wq, np.float64)
    wk = np.asarray(Wk, np.float64)
    wv = np.asarray(Wv, np.float64)
    wo = np.asarray(Wo, np.float64)

    wqk = ((wq.T @ wk) / SCALE).astype(np.float32)
    qt = (A @ wqk).astype(np.float16)
    gate = 1.0 / (
        1.0 + np.exp(-(A @ np.asarray(Wg, np.float32).T + np.asarray(bg, np.float32)))
    )
    gg = (gate * np.asarray(gamma, np.float32)).astype(np.float16)
    gb = (gate * np.asarray(beta, np.float32)).astype(np.float16)
    wvo = np.ascontiguousarray((wo @ wv).T.astype(np.float16))
    m16 = M.astype(np.float16)

    bo_r = np.asarray(bo, np.float32).astype(np.float16).reshape(1, D)
    ones_r = np.ones((1, D), np.float16)

    in_maps = []
    for c in range(NCORES):
        sl = slice(c * BLOC, (c + 1) * BLOC)
        in_maps.append(
            {
                "m": np.ascontiguousarray(m16[sl]),
                "qt": np.ascontiguousarray(qt[sl]),
                "gg": np.ascontiguousarray(gg[sl]),
                "gb": np.ascontiguousarray(gb[sl]),
                "wvo": wvo,
                "ones": ones_r,
                "bo": bo_r,
            }
        )
    return in_maps


def kernel(**inputs) -> np.ndarray:
    nc = _get_program()
    in_maps = make_in_maps(**inputs)
    res = run_bass_kernel_spmd(nc, in_maps, core_ids=list(range(NCORES)))
    return np.concatenate([r["out"] for r in res.results], axis=0).astype(np.float32)


# revision 8
# speedup vs baseline: 1.6255x; 1.1006x over previous
"""Trainium2 Bass kernel for a message-aggregation (single-query attention) block.

Computation (per batch row b):
    Q = A @ Wq.T ; K = M @ Wk.T ; V = M @ Wv.T
    attn = softmax(Q . K / sqrt(D))
    out = sigmoid(A @ Wg.T + bg) * LN(attn-weighted V @ Wo.T + bo)

Host-side algebraic restructuring (exact up to fp reassociation):
    scores[b,n] = Qt[b] . M[b,n]          with Qt = A @ (Wq.T @ Wk) / sqrt(D)
    agg[b]      = (sum_n attn[b,n] M[b,n]) @ (Wo @ Wv).T + bo
    out         = gg * LN_nogamma(agg) + gb   with gg = gate*gamma, gb = gate*beta
Qt, gg, gb are cheap O(B*D) host precomputes; K and V are never materialized.
Messages are shipped to the device in fp16 (halves the HBM stream; score and
attention-weighted sums still accumulate in fp32).

Device dataflow per 128-row batch tile (three-stage software pipeline):
  scores(i):  messages stream in 16-message units; unit 0's scores via fused
              DVE scalar_tensor_tensor (multiply + fp32-accumulated sum in one
              pass), unit 1 via 2x-mode fp16 multiplies + ScalarE
              Copy-with-accumulate reductions (balances DVE vs ScalarE);
              exp (no max subtraction; scores ~ N(0,1)) + its sum in one
              ScalarE instruction.
  accum(i-1): diag(exp) matrices built 16-at-a-time on GPSIMD (broadcast
              access patterns), accumulated into PSUM via TensorE diag
              matmuls (fp16, fp32 accumulation).
  tail(i-2):  1/sumexp folded into the PSUM evacuation, transpose, (Wo Wv).T
              matmul + bias, LayerNorm, gate multiply (GPSIMD), fp16 store
              (upcast on host).

Sharding: pure data parallel over the batch dim across 8 cores; the small
512x512 weights are replicated.
"""

import math
from contextlib import ExitStack

import numpy as np

import concourse.bacc as bacc
import concourse.bass as bass
import concourse.mybir as mybir
import concourse.tile as tile
from concourse.bass_utils import run_bass_kernel_spmd
from concourse.masks import make_identity

B = 4096
N = 32
D = 512
NCORES = 8
BLOC = B // NCORES  # 512
P = 128
NT = BLOC // P  # 4 batch tiles per core
KT = D // P  # 4 contraction tiles
U = 16  # messages per DMA unit / score sub-block
NU = N // U  # 2 units per tile
SCALE = math.sqrt(D)
LN_EPS = 1e-5

F32 = mybir.dt.float32
F16 = mybir.dt.float16
ALU = mybir.AluOpType
ACTF = mybir.ActivationFunctionType


def broadcast_mid(ap2d, count):
    """[P, X] AP -> [P, count, X] AP with a step-0 middle dim."""
    return bass.AP(
        tensor=ap2d.tensor,
        offset=ap2d.offset,
        ap=[ap2d.ap[0], [0, count], ap2d.ap[1]],
    )


def broadcast_last(ap2d, count):
    """[P, X] AP -> [P, X, count] AP with a step-0 last dim."""
    return bass.AP(
        tensor=ap2d.tensor,
        offset=ap2d.offset,
        ap=[ap2d.ap[0], ap2d.ap[1], [0, count]],
    )


def build_program():
    nc = bacc.Bacc(
        "TRN2",
        target_bir_lowering=False,
        debug=False,
        num_devices=NCORES,
    )

    m_d = nc.dram_tensor("m", [BLOC, N, D], F16, kind="ExternalInput")
    qt_d = nc.dram_tensor("qt", [BLOC, D], F16, kind="ExternalInput")
    gg_d = nc.dram_tensor("gg", [BLOC, D], F16, kind="ExternalInput")
    gb_d = nc.dram_tensor("gb", [BLOC, D], F16, kind="ExternalInput")
    wvo_d = nc.dram_tensor("wvo", [D, D], F16, kind="ExternalInput")  # (Wo @ Wv).T
    ones_d = nc.dram_tensor("ones", [1, D], F16, kind="ExternalInput")
    bo_d = nc.dram_tensor("bo", [1, D], F16, kind="ExternalInput")
    out_d = nc.dram_tensor("out", [BLOC, D], F16, kind="ExternalOutput")

    with tile.TileContext(nc) as tc, ExitStack() as ctx:
        consts = ctx.enter_context(tc.tile_pool(name="consts", bufs=1))
        mpool = ctx.enter_context(tc.tile_pool(name="mpool", bufs=6))
        qtp = ctx.enter_context(tc.tile_pool(name="qtp", bufs=NT))
        ggp = ctx.enter_context(tc.tile_pool(name="ggp", bufs=2 * NT))
        wts = ctx.enter_context(tc.tile_pool(name="wts", bufs=KT))
        prodp = ctx.enter_context(tc.tile_pool(name="prodp", bufs=4))
        prod16p = ctx.enter_context(tc.tile_pool(name="prod16p", bufs=4))
        dumpp = ctx.enter_context(tc.tile_pool(name="dumpp", bufs=2))
        scp = ctx.enter_context(tc.tile_pool(name="scp", bufs=3))
        diagp = ctx.enter_context(tc.tile_pool(name="diagp", bufs=4))
        smalls = ctx.enter_context(tc.tile_pool(name="smalls", bufs=3))
        bigp = ctx.enter_context(tc.tile_pool(name="bigp", bufs=2))
        lhstp = ctx.enter_context(tc.tile_pool(name="lhstp", bufs=2))
        outp = ctx.enter_context(tc.tile_pool(name="outp", bufs=2))
        ps_m = ctx.enter_context(tc.tile_pool(name="ps_m", bufs=2, space="PSUM"))
        ps_t = ctx.enter_context(tc.tile_pool(name="ps_t", bufs=2, space="PSUM"))
        ps_a = ctx.enter_context(tc.tile_pool(name="ps_a", bufs=2, space="PSUM"))

        # ---- message stream: issue all DMAs up front (buffer-recycle sems
        # gate the later units); sync/HWDGE ring is dedicated to messages +
        # the output stores.
        m_tiles = []  # [tile][unit]
        for i in range(NT):
            row = []
            for u in range(NU):
                t = mpool.tile([P, U, D], F16, tag="m")
                nc.sync.dma_start(
                    out=t[:],
                    in_=m_d[i * P : (i + 1) * P, u * U : (u + 1) * U, :],
                )
                row.append(t)
            m_tiles.append(row)

        # ---- small inputs on the scalar/HWDGE ring (doesn't queue behind
        # the message stream).
        qt_t = []
        for i in range(NT):
            t = qtp.tile([P, D], F16, tag="qt")
            nc.scalar.dma_start(out=t[:], in_=qt_d[i * P : (i + 1) * P, :])
            qt_t.append(t)

        wvo_t = []
        for k in range(KT):
            t = wts.tile([P, D], F16, tag="w")
            nc.scalar.dma_start(out=t[:], in_=wvo_d[k * P : (k + 1) * P, :])
            wvo_t.append(t)

        ones_row = consts.tile([1, D], F16)
        nc.scalar.dma_start(out=ones_row[:], in_=ones_d[:, :])
        bo_row = consts.tile([1, D], F16)
        nc.scalar.dma_start(out=bo_row[:], in_=bo_d[:, :])

        gg_t = []
        gb_t = []
        for i in range(NT):
            t = ggp.tile([P, D], F16, tag="gg")
            nc.scalar.dma_start(out=t[:], in_=gg_d[i * P : (i + 1) * P, :])
            gg_t.append(t)
            t = ggp.tile([P, D], F16, tag="gb")
            nc.scalar.dma_start(out=t[:], in_=gb_d[i * P : (i + 1) * P, :])
            gb_t.append(t)

        # ---- constants ------------------------------------------------
        ident = consts.tile([P, P], F16)
        make_identity(nc, ident[:])
        eps_t = consts.tile([P, 1], F32)
        nc.vector.memset(eps_t[:], LN_EPS)

        # ---- pipeline stages ------------------------------------------
        def emit_scores(i):
            """Fill sc[P, N] (fp32) and expd[P, N] (fp16) + sumexp."""
            sc = scp.tile([P, N], F32, tag="sc")
            # unit 1 multiplies first (2x mode) so ScalarE can start reducing
            # while DVE works through unit 0's fused score ops.
            mt1 = m_tiles[i][1]
            prods = []
            for h in range(2):
                p16 = prod16p.tile([P, U // 2, D], F16, tag="prod16")
                nc.vector.tensor_mul(
                    p16[:],
                    mt1[:, h * (U // 2) : (h + 1) * (U // 2), :],
                    broadcast_mid(qt_t[i][:], U // 2),
                )
                prods.append(p16)
            for j in range(U):
                h, jj = divmod(j, U // 2)
                dump = dumpp.tile([P, D], F16, tag="dump")
                nc.scalar.activation(
                    dump[:],
                    prods[h][:, jj, :],
                    ACTF.Copy,
                    accum_out=sc[:, U + j : U + j + 1],
                )
            # unit 0: fused multiply+reduce on DVE, one instruction/message
            mt0 = m_tiles[i][0]
            for j in range(U):
                prod = prodp.tile([P, D], F16, tag="prod")
                nc.vector.scalar_tensor_tensor(
                    out=prod[:],
                    in0=mt0[:, j, :],
                    scalar=0.0,
                    in1=qt_t[i][:],
                    op0=ALU.bypass,
                    op1=ALU.mult,
                    accum_out=sc[:, j : j + 1],
                )
            # exp (no max subtraction) + softmax denominator; per-unit so the
            # diag build of unit 0 can start before unit 1's scores finish
            expd = scp.tile([P, N], F16, tag="expd")
            separts = smalls.tile([P, 2], F32, tag="separts")
            nc.scalar.activation(
                expd[:, 0:U], sc[:, 0:U], ACTF.Exp, accum_out=separts[:, 0:1]
            )
            nc.scalar.activation(
                expd[:, U:N], sc[:, U:N], ACTF.Exp, accum_out=separts[:, 1:2]
            )
            sumexp = smalls.tile([P, 1], F32, tag="sumexp")
            nc.vector.tensor_tensor(
                sumexp[:], separts[:, 0:1], separts[:, 1:2], op=ALU.add
            )
            return expd, sumexp

        def emit_accum(i, state):
            """exp-weighted message sum into PSUM via diag matmuls."""
            expd, _ = state
            pm = ps_m.tile([P, D], F32, tag="pm")
            for u in range(NU):
                # dg[p, n, j] = (p == j) ? expd[p, n] : 0 — one GPSIMD pass,
                # single read stream (no SBUF-port fight with DVE's 2-read ops)
                dg = diagp.tile([P, U, P], F16, tag="diag")
                nc.gpsimd.affine_select(
                    out=dg[:],
                    in_=broadcast_last(expd[:, u * U : (u + 1) * U], P),
                    compare_op=ALU.is_equal,
                    fill=0.0,
                    base=0,
                    pattern=[[0, U], [-1, P]],
                    channel_multiplier=1,
                )
                for j in range(U):
                    n = u * U + j
                    nc.tensor.matmul(
                        pm[:],
                        lhsT=dg[:, j, :],
                        rhs=m_tiles[i][u][:, j, :],
                        start=(n == 0),
                        stop=(n == N - 1),
                    )
            return pm

        def emit_tail(i, state, pm):
            expd, sumexp = state
            rsum = smalls.tile([P, 1], F32, tag="rsum")
            nc.vector.reciprocal(rsum[:], sumexp[:])
            # fold 1/sum(exp) into the PSUM evacuation
            magg = bigp.tile([P, D], F16, tag="magg")
            nc.scalar.mul(magg[:], pm[:], rsum[:, 0:1])

            # transpose m_agg so it can be the stationary operand
            pt = ps_t.tile([P, KT, P], F16, tag="pt")
            for j in range(KT):
                nc.tensor.transpose(pt[:, j, :], magg[:, j * P : (j + 1) * P], ident[:])
            maggT = lhstp.tile([P, KT, P], F16, tag="lhst")
            for j in range(KT):
                nc.scalar.copy(maggT[:, j, :], pt[:, j, :])

            # agg = m_agg @ (Wo Wv).T + bo
            pa = ps_a.tile([P, D], F32, tag="pa")
            for j in range(KT):
                nc.tensor.matmul(
                    pa[:],
                    lhsT=maggT[:, j, :],
                    rhs=wvo_t[j][:],
                    start=(j == 0),
                    stop=False,
                )
            nc.tensor.matmul(
                pa[:],
                lhsT=ones_row[:, 0:P],
                rhs=bo_row[:],
                start=False,
                stop=True,
            )

            # LayerNorm over d (gamma/beta folded into gg/gb on host)
            stats = smalls.tile([P, nc.vector.BN_STATS_DIM], F32, tag="stats")
            nc.vector.bn_stats(stats[:], pa[:])
            mv = smalls.tile([P, nc.vector.BN_AGGR_DIM], F32, tag="mv")
            nc.vector.bn_aggr(mv[:], stats[:])
            sq = smalls.tile([P, 1], F32, tag="sq")
            nc.scalar.activation(sq[:], mv[:, 1:2], ACTF.Sqrt, bias=eps_t[:, 0:1])
            rstd = smalls.tile([P, 1], F32, tag="rstd")
            nc.vector.reciprocal(rstd[:], sq[:])
            negmr = smalls.tile([P, 1], F32, tag="negmr")
            nc.vector.tensor_scalar(
                negmr[:],
                mv[:, 0:1],
                scalar1=rstd[:, 0:1],
                scalar2=-1.0,
                op0=ALU.mult,
                op1=ALU.mult,
            )
            normed = outp.tile([P, D], F16, tag="normed")
            nc.scalar.activation(
                normed[:], pa[:], ACTF.Identity, bias=negmr[:, 0:1], scale=rstd[:, 0:1]
            )

            # out = gg*normed + gb  (fp16 2x-mode DVE ops, ~0.9us per tile)
            o = outp.tile([P, D], F16, tag="out")
            nc.vector.tensor_mul(o[:], normed[:], gg_t[i][:])
            nc.vector.tensor_add(o[:], o[:], gb_t[i][:])
            nc.sync.dma_start(out=out_d[i * P : (i + 1) * P, :], in_=o[:])

        states = [None] * NT
        pms = [None] * NT
        for s in range(NT + 2):
            if s < NT:
                states[s] = emit_scores(s)
            if 0 <= s - 1 < NT:
                pms[s - 1] = emit_accum(s - 1, states[s - 1])
            if 0 <= s - 2 < NT:
                emit_tail(s - 2, states[s - 2], pms[s - 2])

    nc.compile()
    return nc


_CACHED_NC = None


def _get_program():
    global _CACHED_NC
    if _CACHED_NC is None:
        _CACHED_NC = build_program()
    return _CACHED_NC


def make_in_maps(agent_hidden, messages, Wq, Wk, Wv, Wo, bo, gamma, beta, Wg, bg):
    A = np.asarray(agent_hidden, np.float32)
    M = np.asarray(messages, np.float32)
    wq = np.asarray(Wq, np.float64)
    wk = np.asarray(Wk, np.float64)
    wv = np.asarray(Wv, np.float64)
    wo = np.asarray(Wo, np.float64)

    wqk = ((wq.T @ wk) / SCALE).astype(np.float32)
    qt = (A @ wqk).astype(np.float16)
    gate = 1.0 / (
        1.0 + np.exp(-(A @ np.asarray(Wg, np.float32).T + np.asarray(bg, np.float32)))
    )
    gg = (gate * np.asarray(gamma, np.float32)).astype(np.float16)
    gb = (gate * np.asarray(beta, np.float32)).astype(np.float16)
    wvo = np.ascontiguousarray((wo @ wv).T.astype(np.float16))
    m16 = M.astype(np.float16)

    bo_r = np.asarray(bo, np.float32).astype(np.float16).reshape(1, D)
    ones_r = np.ones((1, D), np.float16)

    in_maps = []
    for c in range(NCORES):
        sl = slice(c * BLOC, (c + 1) * BLOC)
        in_maps.append(
            {
                "m": np.ascontiguousarray(m16[sl]),
                "qt": np.ascontiguousarray(qt[sl]),
                "gg": np.ascontiguousarray(gg[sl]),
                "gb": np.ascontiguousarray(gb[sl]),
                "wvo": wvo,
                "ones": ones_r,
                "bo": bo_r,
            }
        )
    return in_maps


def kernel(**inputs) -> np.ndarray:
    nc = _get_program()
    in_maps = make_in_maps(**inputs)
    res = run_bass_kernel_spmd(nc, in_maps, core_ids=list(range(NCORES)))
    return np.concatenate([r["out"] for r in res.results], axis=0).astype(np.float32)
